# revision 30
# baseline (speedup 1.0000x reference)
"""Causal multi-head attention block on 8 TRN2 NeuronCores.

Sharding: tensor-parallel over heads (16 heads -> 2 per core) for QKV +
attention; AllToAll switches to token-parallel for the output projection.

Per-core device graph (all bf16 matmuls, fp32 PSUM accumulation):
  1. QKV: q^T,k^T in [e,t] layout (e = 2 heads x 64 on partitions),
     v in [t,e] layout, from a resident x^T [1024, 4096].
  2. Attention per (batch, head): scores computed TRANSPOSED
     s^T[kv, q] = k^T.T @ q^T; the softmax denominator comes free from a
     ones-column appended to v (rows of attn^T psum: 0:64 = numerator,
     64 = denom). No max-subtraction (scores ~ N(0,1) after folding
     1/sqrt(dk) into w_q on the host; exp can't overflow).
  3. Normalize via fast reciprocal of a DMA partition-broadcast of the
     denominator row, stage to a DRAM AllToAll buffer as bf16.
  4. AllToAll (head-shards -> token-shards), then out = attn^T.T @ w_p^T
     per 128-token chunk, + host-folded constant bias vector.

Scheduling structure:
  - QKV is a need-driven generator woven into the attention emission so
    the PE always has queued matmuls while ACT runs exp.
  - The causal mask is applied on the PE (ident.T @ lower_tri(-60)
    accumulated into score PSUM) so exp needs no second sync wait.
  - The AllToAll is split into EIGHT per-(batch,group) 128KB collectives
    so all but the last overlap attention compute; the last one's
    latency is covered by projecting the already-delivered chunks.
    Token ownership is interleaved: each core's 512-token chunk is 64
    tokens from each (batch, query-group) window, so every attention
    group feeds one collective the moment it finishes.
  - Bulk loads (x, weights, A2A staging/unstaging) are single
    multi-dim-AP DMA instructions, not per-tile loops: DMA_DIRECT2D
    issue costs ~0.6us on the sync engine, and the old per-tile loops
    put ~86us of issue work (and head-of-line stalls at the tail) on
    that queue.
"""

import numpy as np
import ml_dtypes

import concourse.bass as bass
import concourse.bacc as bacc
import concourse.mybir as mybir
from concourse.tile import TileContext, add_dep_helper
from concourse.bass_utils import run_bass_kernel_spmd

NC = 8                      # cores
B, S, D = 2, 2048, 1024
H, DK = 16, 64
HPC = H // NC               # heads per core = 2
EC = HPC * DK               # embed dims per core = 128
T = B * S                   # 4096 flattened tokens
TC = T // NC                # tokens per core chunk = 512
K8 = D // 128               # contraction tiles = 8
SCALE = 1.0 / np.sqrt(DK)

BF16 = mybir.dt.bfloat16
F32 = mybir.dt.float32
NPBF16 = ml_dtypes.bfloat16

_CACHE = {}
# tail choreography variant (A/B tested; "v10" = delayed ti1/ti2 won)
_TAIL = "v10"


def _build_nc(dbg: bool = False) -> bass.Bass:
    nc = bacc.Bacc("TRN2", target_bir_lowering=False, debug=False, num_devices=NC)

    xT = nc.declare_dram_parameter("xT", [D, T], BF16, isOutput=False)
    wqkvT = nc.declare_dram_parameter("wqkvT", [D, 3 * EC], BF16, isOutput=False)
    wpT = nc.declare_dram_parameter("wpT", [D, D], BF16, isOutput=False)
    cvec = nc.declare_dram_parameter("cvec", [1, D], F32, isOutput=False)
    out = nc.declare_dram_parameter("out", [TC, D], F32, isOutput=True)

    # causal masking happens on the PE: a matmul accumulates
    # ident.T @ maskneg = -60 on the strict lower triangle (kv > q) into
    # the diagonal score tiles, so exp() itself produces ~0 there and no
    # vector-engine op (and no extra ACT sync wait) is needed.
    ident_np = np.eye(128, dtype=NPBF16)
    maskneg_np = np.where(np.arange(128)[:, None] > np.arange(128)[None, :],
                          -60.0, 0.0).astype(NPBF16)
    ident_dram = nc.inline_tensor(ident_np, name="ident128")
    maskneg_dram = nc.inline_tensor(maskneg_np, name="maskneg128")

    xT3 = xT.rearrange("(k p) t -> p k t", k=K8)        # [128, 8, 4096]
    wqkv3 = wqkvT.rearrange("(k p) e -> p k e", k=K8)   # [128, 8, 384]
    wp3 = wpT.rearrange("(k p) e -> p k e", k=K8)       # [128, 8, 1024]

    with TileContext(nc) as tc:
        with (
            tc.tile_pool(name="const", bufs=1) as constp,
            tc.tile_pool(name="x", bufs=1) as xp,
            tc.tile_pool(name="qk", bufs=1) as qkp,
            tc.tile_pool(name="w", bufs=1) as wp,
            tc.tile_pool(name="ps", bufs=8, space="PSUM") as psp,
            tc.tile_pool(name="pt", bufs=6) as ptp,
            tc.tile_pool(name="nrm", bufs=2) as nrmp,
            tc.tile_pool(name="stage", bufs=4) as stp,
            tc.tile_pool(name="dram", bufs=1, space="DRAM") as dramp,
            tc.tile_pool(name="proj", bufs=1) as projp,
        ):
            # ---- constants ----
            ident_sb = constp.tile([128, 128], BF16)
            nc.sync.dma_start(out=ident_sb[:, :], in_=ident_dram[:, :])
            maskneg_sb = constp.tile([128, 128], BF16)
            nc.sync.dma_start(out=maskneg_sb[:, :], in_=maskneg_dram[:, :])
            cv_ld = constp.tile([128, D], F32)
            nc.gpsimd.dma_start(out=cv_ld[:, :], in_=cvec[:, :].to_broadcast([128, D]))
            cv_b = constp.tile([128, D], F32)
            nc.vector.tensor_copy(cv_b[:, :], cv_ld[:, :])
            # touch Exp early so the ~2.7us ACT table load hides in the
            # preamble instead of stalling the first real softmax tile
            actwarm = constp.tile([1, 32], BF16)
            nc.scalar.activation(actwarm[:, :], cv_b[0:1, 0:32],
                                 mybir.ActivationFunctionType.Exp)

            # ---- load x^T and weights ----
            # The first QKV accumulation chain needs wqkv and x[:, 0:512]
            # for all k-tiles; issue exactly that first, per-k so tiles
            # land incrementally, spread over four issuing engines (a
            # DMA_DIRECT2D issue costs ~0.6us, serial on one engine).
            x_sb = xp.tile([128, K8, T], BF16)          # 8 MB
            wqkv_sb = wp.tile([128, K8, 3 * EC], BF16)
            issuers = [nc.sync, nc.gpsimd, nc.scalar]
            for k in range(K8):
                issuers[k % 3].dma_start(
                    out=wqkv_sb[:, k, :], in_=wqkv3[:, k, :])
                issuers[(k + 1) % 3].dma_start(
                    out=x_sb[:, k, 0:512], in_=xT3[:, k, 0:512])
            nc.sync.dma_start(out=x_sb[:, :, 512:1024], in_=xT3[:, :, 512:1024])
            for c in range(1, 4):
                nc.sync.dma_start(
                    out=x_sb[:, :, c * 1024:(c + 1) * 1024],
                    in_=xT3[:, :, c * 1024:(c + 1) * 1024],
                )
            wp_sb = wp.tile([128, K8, D], BF16)         # w_proj^T (loaded mid-kernel)

            q_sb = qkp.tile([EC, T], BF16)
            k_sb = qkp.tile([EC, T], BF16)
            # v layout: per 128-token tile, [v_h0(64) | ones | v_h1(64) | ones]
            v_sb = qkp.tile([128, T // 128, 130], BF16)
            nc.vector.memset(v_sb[:, :, 64:65], 1.0)    # ones column, head 0
            nc.vector.memset(v_sb[:, :, 129:130], 1.0)  # ones column, head 1

            # PSUM tag budget (8 banks): pso 3 + pss 2x2 + fill 1 = 8.
            # QKV is emitted as a need-driven filler stream woven into the
            # attention loops, so the PE has dense queued work while ACT
            # runs exp. A progress ledger guarantees every q/k/v tile's
            # producing copy is emitted before any consumer.
            prog = {("q", 0): 0, ("k", 0): 0, ("v", 0): 0,
                    ("q", 1): 0, ("k", 1): 0, ("v", 1): 0}

            def qkv_stream(b):
                """Emit batch b's QKV, one instruction per next()."""
                base = b * S

                def emit_qk(sec, n):
                    pq = psp.tile([128, 512], F32, tag="fill", bufs=1,
                                  name=f"f{b}{sec}{n}")
                    for k in range(K8):
                        nc.tensor.matmul(
                            pq[:, :],
                            lhsT=wqkv_sb[:, k, sec * EC:(sec + 1) * EC],
                            rhs=x_sb[:, k, base + n * 512:base + (n + 1) * 512],
                            start=(k == 0), stop=(k == K8 - 1),
                        )
                        yield
                    dst = q_sb if sec == 0 else k_sb
                    nc.vector.tensor_copy(
                        dst[:, base + n * 512:base + (n + 1) * 512], pq[:, :]
                    )
                    prog[("qk"[sec], b)] = (n + 1) * 512
                    yield

                def emit_v(tq):
                    pv = psp.tile([128, 4, 128], F32, tag="fill", bufs=1,
                                  name=f"fv{b}{tq}")
                    for k in range(K8):
                        for t2 in range(4):
                            nc.tensor.matmul(
                                pv[:, t2, :],
                                lhsT=x_sb[:, k, base + (tq * 4 + t2) * 128:
                                          base + (tq * 4 + t2 + 1) * 128],
                                rhs=wqkv_sb[:, k, 2 * EC:3 * EC],
                                start=(k == 0 and t2 == 0), stop=(k == K8 - 1),
                            )
                            yield
                    src4 = pv[:, :, :].rearrange("p t (h e) -> p t h e", h=2)
                    dst4 = v_sb[:, b * 16 + tq * 4:b * 16 + (tq + 1) * 4, :]\
                        .rearrange("p t (h e) -> p t h e", e=65)[:, :, :, 0:64]
                    nc.vector.tensor_copy(dst4, src4)
                    prog[("v", b)] = (tq + 1) * 4
                    yield

                for n in range(4):
                    yield from emit_qk(0, n)
                    yield from emit_qk(1, n)
                    yield from emit_v(n)

            def drain(it, n):
                for _ in range(n):
                    try:
                        next(it)
                    except StopIteration:
                        return

            # Per-(batch,group) A2A stages. Core c owns tokens
            # [64c, 64c+64) of every group's 512-token window; its output
            # chunk is the concatenation over the 8 windows.
            a2a_in = [dramp.tile([NC, 128, 64], BF16, name=f"a2ai{s}")
                      for s in range(8)]
            a2a_out = [dramp.tile([NC, 128, 64], BF16, name=f"a2ao{s}")
                       for s in range(8)]

            last_av = [None]
            last_mul = [None]

            def emit_attention(b, fill, fpi):
                """Attention for batch b; `fill` instructions are woven in
                (need-driven + `fpi` extra per kv-tile) to keep the PE dense
                while ACT runs exp. Both heads share one 2-bank score tile so
                exp is a single ACT instruction per kv-tile."""
                for g in range(S // 512):               # 4 query groups
                    pso = [psp.tile([128, 512], F32, tag="pso", bufs=3,
                                    name=f"pso_{b}_{g}_{hi}") for hi in range(2)]
                    nkv = 4 * (g + 1)
                    prev = None
                    for kj in range(nkv):
                        # producers of this iteration's q/k/v tiles must be
                        # emitted first (correctness under tile-granular
                        # dependency tracking, not just performance)
                        while (prog[("k", b)] < (kj + 1) * 128
                               or prog[("q", b)] < (g + 1) * 512
                               or prog[("v", b)] < kj + 1):
                            next(fill)
                        d = kj - 4 * g                  # >=0 on diagonal band
                        j0 = 128 * d if d >= 0 else 0
                        pss2 = psp.tile([128, 2, 512], F32, tag="pss", bufs=2,
                                        name=f"pss_{b}_{g}_{kj}")
                        for hi in range(2):
                            h0 = hi * DK
                            nc.tensor.matmul(
                                pss2[:, hi, j0:512],
                                lhsT=k_sb[h0:h0 + DK,
                                          b * S + kj * 128:b * S + (kj + 1) * 128],
                                rhs=q_sb[h0:h0 + DK,
                                         b * S + g * 512 + j0:b * S + (g + 1) * 512],
                                start=True, stop=(d < 0),
                            )
                            if d >= 0:
                                nc.tensor.matmul(
                                    pss2[:, hi, j0:j0 + 128],
                                    lhsT=ident_sb[:, :], rhs=maskneg_sb[:, :],
                                    start=False, stop=True,
                                )
                        pt2 = ptp.tile([128, 2, 512], BF16, tag="pt",
                                       name=f"pt_{b}_{g}_{kj}")
                        nc.scalar.activation(
                            pt2[:, :, j0:512], pss2[:, :, j0:512],
                            mybir.ActivationFunctionType.Exp,
                        )
                        if prev is not None:
                            ppt, pj0, pkj = prev
                            for hi in range(2):
                                nc.tensor.matmul(
                                    pso[hi][:65, pj0:512],
                                    lhsT=v_sb[:, (b * S) // 128 + pkj,
                                              hi * 65:(hi + 1) * 65],
                                    rhs=ppt[:, hi, pj0:512],
                                    start=(pkj == 0), stop=False,
                                )
                        drain(fill, fpi + (4 if g == 0 else 0))
                        prev = (pt2, j0, kj)
                    ppt, pj0, pkj = prev
                    for hi in range(2):
                        last_av[0] = nc.tensor.matmul(
                            pso[hi][:65, pj0:512],
                            lhsT=v_sb[:, (b * S) // 128 + pkj,
                                      hi * 65:(hi + 1) * 65],
                            rhs=ppt[:, hi, pj0:512],
                            start=(pkj == 0), stop=True,
                        )
                    # normalize rows 0:64 by row 64, stage for A2A
                    sidx = b * 4 + g
                    for hi in range(2):
                        dn = nrmp.tile([1, 512], F32, tag="dn")
                        nc.vector.tensor_copy(dn[:, :], pso[hi][64:65, :])
                        dnb = nrmp.tile([64, 512], F32, tag="dnb")
                        nc.gpsimd.partition_broadcast(dnb[:, :], dn[:1, :])
                        rb = nrmp.tile([64, 512], F32, tag="rb")
                        nc.vector.reciprocal_approx_fast(rb[:, :], dnb[:, :])
                        aout = stp.tile([64, 512], BF16, tag="aout", bufs=3)
                        last_mul[0] = nc.vector.tensor_mul(
                            aout[:, :], pso[hi][0:64, :], rb[:, :])
                        # scatter my 512 q-columns to their 8 owner cores
                        nc.sync.dma_start(
                            out=a2a_in[sidx][:, hi * 64:(hi + 1) * 64, :]
                                .rearrange("c p t -> p c t"),
                            in_=aout[:, :].rearrange("p (c t) -> p c t", c=NC),
                        )
                    nc.gpsimd.collective_compute(
                        "AllToAll",
                        mybir.AluOpType.bypass,
                        ins=[a2a_in[sidx].opt()],
                        outs=[a2a_out[sidx].opt()],
                        replica_groups=[list(range(NC))],
                    )

            # one shared QKV filler stream spanning both batches; the
            # need-driven pulls in emit_attention take what they must, fpi
            # paces the rest through PE idle slots.
            import itertools
            fill = itertools.chain(qkv_stream(0), qkv_stream(1))
            emit_attention(0, fill, 4)
            # w_proj load: issued here (sync queue reaches it ~batch-0
            # staging time) so it is resident long before proj, without
            # contending with the startup x loads or getting stuck behind
            # the final staging DMAs.
            nc.sync.dma_start(out=wp_sb[:, :, :], in_=wp3[:, :, :])
            emit_attention(1, fill, 2)

            # ---- output projection on my 512-token chunk ----
            # at_t[j][:, s, :]: partitions = src core s's 128 embed dims
            # (= global dims 128s.., matching wpT rows); free = my 64
            # tokens from stage 2j then stage 2j+1.
            at_t = []
            for j in range(4):
                t = projp.tile([128, NC, 128], BF16, name=f"at{j}")
                for half in range(2):
                    s = 2 * j + half
                    if s == 7:
                        continue          # final-collective load issued late
                    nc.sync.dma_start(
                        out=t[:, :, half * 64:(half + 1) * 64],
                        in_=a2a_out[s].rearrange("s p t -> p s t"),
                    )
                at_t.append(t)

            # Tail choreography. The final collective's wire transfers run
            # at ~6GB/s if the compute engines are idle (HAM clock gating)
            # vs ~37GB/s when busy, so proj must EXECUTE during its window,
            # not before it: ti0 fills the PE right after the last AV
            # (covering the normalize+staging+trigger latency), ti1/ti2 are
            # sync-anchored to the last normalize so they run while the
            # collective's data moves, and ti3 consumes it.
            out_dmas = []
            for ti in range(4):
                pspj = [psp.tile([128, 512], F32, tag="pso", bufs=3,
                                 name=f"pspj_{ti}_{ng}") for ng in range(2)]
                for ng in range(2):
                    for k in range(K8):
                        mm = nc.tensor.matmul(
                            pspj[ng][:, :],
                            lhsT=at_t[ti][:, k, :],
                            rhs=wp_sb[:, k, ng * 512:(ng + 1) * 512],
                            start=(k == 0), stop=(k == K8 - 1),
                        )
                        if k == 0 and ng == 0:
                            add_dep_helper(mm.ins, last_av[0].ins, sync=False,
                                           reason="proj after attention")
                            if _TAIL == "v10" and ti in (1, 2):
                                add_dep_helper(mm.ins, last_mul[0].ins,
                                               sync=True,
                                               reason="run during final cc")
                osb = stp.tile([128, D], F32, tag="osb", bufs=2)
                for ng in range(2):
                    nc.vector.tensor_add(
                        osb[:, ng * 512:(ng + 1) * 512], pspj[ng][:, :],
                        cv_b[:, ng * 512:(ng + 1) * 512],
                    )
                out_dmas.append(nc.sync.dma_start(
                    out=out[ti * 128:(ti + 1) * 128, :], in_=osb[:, :]
                ))
                if ti == (0 if _TAIL == "v10" else 2):
                    # the load gated on the FINAL collective sits on the
                    # sync queue after this tile's out DMA: its wait blocks
                    # the queue only while nothing else needs issuing.
                    ld = nc.sync.dma_start(
                        out=at_t[3][:, :, 64:128],
                        in_=a2a_out[7].rearrange("s p t -> p s t"),
                    )
                    add_dep_helper(ld.ins, out_dmas[-1].ins, sync=False,
                                   reason="cc-gated load late on sync queue")
    nc.compile()
    return nc


def _prep_inputs(x, w_atten, b_atten, w_proj, b_proj):
    x = np.asarray(x, dtype=np.float32)
    w_atten = np.asarray(w_atten, dtype=np.float32)
    b_atten = np.asarray(b_atten, dtype=np.float32)
    w_proj = np.asarray(w_proj, dtype=np.float32)
    b_proj = np.asarray(b_proj, dtype=np.float32)

    xT = np.ascontiguousarray(x.reshape(T, D).T).astype(NPBF16)
    wpT = np.ascontiguousarray(w_proj.T).astype(NPBF16)
    # v-bias routes through softmax as an additive constant: fold into cvec
    cvec = (b_atten[2 * D:3 * D] @ w_proj.T + b_proj).astype(np.float32)[None, :]

    in_maps = []
    for c in range(NC):
        r = slice(c * EC, (c + 1) * EC)
        wq = w_atten[0 * D:1 * D][r] * SCALE     # fold score scale into w_q
        wk = w_atten[1 * D:2 * D][r]
        wv = w_atten[2 * D:3 * D][r]
        wqkvT = np.ascontiguousarray(
            np.concatenate([wq.T, wk.T, wv.T], axis=1)
        ).astype(NPBF16)
        assert np.all(b_atten[:2 * D] == 0.0), "nonzero q/k bias unsupported"
        in_maps.append({
            "xT": xT, "wqkvT": wqkvT, "wpT": wpT,
            "cvec": cvec,
        })
    return in_maps


def _run(inputs: dict, trace: bool = False):
    if "nc" not in _CACHE:
        _CACHE["nc"] = _build_nc()
    nc = _CACHE["nc"]
    in_maps = _prep_inputs(**inputs)
    res = run_bass_kernel_spmd(nc, in_maps, core_ids=list(range(NC)), trace=trace)
    # chunk rows: 8 pieces of 64 tokens; piece s = (batch s//4, group s%4),
    # global tokens [512*(s%4) + 64c, +64) of that batch
    st = np.stack([res.results[c]["out"] for c in range(NC)])  # [NC, 512, D]
    st = st.reshape(NC, 8, 64, D)
    full = np.empty((B, S, D), dtype=np.float32)
    for b in range(B):
        for g in range(4):
            # [NC, 64, D] -> tokens of group-g window, core-major
            full[b, 512 * g:512 * (g + 1)] = st[:, 4 * b + g].reshape(512, D)
    return full, res


def kernel(**inputs) -> np.ndarray:
    out, _ = _run(inputs, trace=False)
    return out
